# revision 6
# baseline (speedup 1.0000x reference)
"""Bahdanau additive attention scores on 8 Trainium2 NeuronCores — v2.

reference:
    a = query @ W2.T                      # [B, Tq, D]
    bk = key @ W1.T + (b1 + b2)           # [B, Tk, D]
    scores[b,q,k] = sum_d v[d] * tanh(a[b,q,d] + bk[b,k,d])

Approximation: tanh(x) ~= sum_i c_i sin(om_i x) with om_i from a
frequency-doubling ladder {0.49, 0.98, 1.96, 3.92} (ridge-fit to the
Gaussian-weighted data range; end-to-end rel err ~5.5e-3, well under the
2e-2 gate).  Each sine of a sum splits separably:
    sin(om(a+b)) = sin(om a)cos(om b) + cos(om a)sin(om b)
so scores become 2*F=8 rank-D matmul accumulations per chunk — PE work —
and the only elementwise work is per-side sin/cos at 4 frequencies.

Per-side tables are built WITHOUT range reduction (the DVE has no float mod):
  - base sin/cos via one ACT Sin each (args |om0 x| <= 1.71; the cos
    phase-shift +pi/2 keeps args <= 3.28, inside the HW table's accurate
    range, measured good to ~3.3),
  - each doubling level: tau = sigma_{l-1}^2 (ACT Square),
    sigma_l = sigma_{l-1} * c_{l-1} (DVE tensor_tensor),
    c_l = 1 - 2^(2l-1) tau (Pool tensor_scalar).
    sigma_l stores sin/2^l; the 2^l rides the fitted coefficient.

The k-side bias (b1+b2) is added inside the PE projection via a K=1 matmul
(bias row x ones row), so the k-side Sin instructions batch all 4 feature
chunks.  The input rides two parallel DMA queues (q-projection operands on
one, k-side on the other).  Score matmuls are issued per ladder level, so
level-l matmuls overlap the level-l+1 chain work.

Sharding: data-parallel over batch B=8 -> 1 batch per core; weights
replicated.  D=512 is split into 4 partition chunks of 128; PE projects
q/k with f16 weights straight into PSUM and ACT reads PSUM directly.
"""

import sys

sys.path.insert(0, "/opt/trn_rl_repo")

import numpy as np

B, TQ, TK, D = 8, 128, 256, 512
NC_ = 4  # 128-wide feature chunks
BASES = (0.49,)
NLVLS = (4,)  # ladder levels per base (freqs base * 2^l)
NLAD = len(BASES)
# ridge fit (alpha=3e-3, weighted by empirical |a+b| density, S=6.12):
COEF = ((1.06213, 0.17335, 0.13537, 0.01141),)
HALF_PI = float(np.pi / 2)

_cache = {}


def _patch_drain(tile):
    """Split kernel-tail drain waits over several SP no-ops (<=3 waits each) to
    stay within the ISA's per-instruction sync-wait budget."""
    from concourse.vector_clock import VectorClock, ScopedClock

    if getattr(tile.TileContext, "_drain_patched", False):
        return

    def _drain_and_barrier(self, tick_clock, wait_clock):
        gc = tick_clock.global_clock
        pending = [(p, t) for p, t in enumerate(gc) if t > 0]
        for i in range(0, len(pending), 1):
            chunk = pending[i : i + 1]
            nop = self.nc.sync.nop(nofuse=True, hint="drain_split")
            req = VectorClock()
            for p, t in chunk:
                req.require_at_least(p, t)
            wait_clock.add_sem_waits(nop.ins, ScopedClock({None: req}))
        self.nc.sync.drain()
        self.nc.all_engine_barrier()
        assert self.sems is not None
        popped = self.nc._tile_sem_poison_stack.pop()
        assert popped is self._sem_poison
        self.nc.clear_and_free_semaphores(list(self.sems.allocated().values()))
        self.nc.all_engine_barrier()

    tile.TileContext._drain_and_barrier = _drain_and_barrier
    tile.TileContext._drain_patched = True


# packed-input layout, two DMA halves (f32 word columns)
# half A (q-projection + scalars): qT (256), w2T (1024), scales (2), hp (1),
#   v (4)  -> 1287 cols
# half B (k-side): kT (512), w1T (1024), brow (256), ones (128) -> 1920 cols
_A_QT = 0
_A_W2 = _A_QT + 256
_A_SCALE = _A_W2 + 1024
_A_HP = _A_SCALE + 2
_A_V = _A_HP + 1
_A_Z = _A_V + 4  # zero col (explicit activation bias; const-AP table is skipped)
_A_ONE = _A_Z + 1  # one col (explicit activation scale)
_NCOLA = _A_ONE + 1
_B_KT = 0
_B_W1 = _B_KT + 512
_B_BROW = _B_W1 + 1024
_B_ONES = _B_BROW + 256
_NCOLB = _B_ONES + 128


def _build_nc():
    import concourse.bass as bass
    import concourse.mybir as mybir
    import concourse.tile as tile

    _patch_drain(tile)
    f32 = mybir.dt.float32
    f16 = mybir.dt.float16
    Alu = mybir.AluOpType
    Act = mybir.ActivationFunctionType

    # Skip the const-AP memsets and the init all-engine barrier: every
    # activation below passes explicit scale/bias column APs, so the const-AP
    # table is never read, and removing the barrier lets the input DMA issue
    # as soon as the SP engine finishes its own bootstrap.
    orig_bar = bass.Bass.all_engine_barrier
    orig_memset = bass.BassSharedVectorInterface.memset
    bass.Bass.all_engine_barrier = lambda self: None
    bass.BassSharedVectorInterface.memset = lambda self, ap, c: None
    try:
        nc = bass.Bass()
    finally:
        bass.Bass.all_engine_barrier = orig_bar
        bass.BassSharedVectorInterface.memset = orig_memset
    inpa = nc.dram_tensor("inpa", [128, _NCOLA], f32, kind="ExternalInput")
    inpb = nc.dram_tensor("inpb", [128, _NCOLB], f32, kind="ExternalInput")
    scores = nc.dram_tensor("scores", [TQ, TK], f32, kind="ExternalOutput")

    with tile.TileContext(nc) as tc:
        with (
            tc.tile_pool(name="consts", bufs=1) as consts,
            tc.tile_pool(name="qproj_ps", bufs=1, space="PSUM") as qproj_ps,
            tc.tile_pool(name="kproj_ps", bufs=1, space="PSUM") as kproj_ps,
            tc.tile_pool(name="score_ps", bufs=1, space="PSUM") as score_ps,
            tc.tile_pool(name="dum_ps", bufs=1, space="PSUM") as dum_ps,
            tc.tile_pool(name="gate_ps", bufs=1, space="PSUM") as gate_ps,
            tc.tile_pool(name="chain", bufs=1) as chain,
            tc.tile_pool(name="scr", bufs=8) as scr,
            tc.tile_pool(name="outp", bufs=1) as outp,
        ):
            ina_sb = consts.tile([128, _NCOLA], f32)
            nc.sync.dma_start(out=ina_sb, in_=inpa[:])
            inb_sb = consts.tile([128, _NCOLB], f32)
            nc.scalar.dma_start(out=inb_sb, in_=inpb[:])

            qT = ina_sb[:, _A_QT : _A_QT + 256].bitcast(f16).rearrange(
                "p (c q) -> p c q", c=NC_
            )
            w2T = ina_sb[:, _A_W2 : _A_W2 + 1024].bitcast(f16).rearrange(
                "p (c o) -> p c o", c=NC_
            )
            scale_cols = ina_sb[:, _A_SCALE : _A_SCALE + 2]
            hp_col = ina_sb[:, _A_HP : _A_HP + 1]
            v_cols = ina_sb[:, _A_V : _A_V + 4]
            z_col = ina_sb[:, _A_Z : _A_Z + 1]
            one_col = ina_sb[:, _A_ONE : _A_ONE + 1]
            kT = inb_sb[:, _B_KT : _B_KT + 512].bitcast(f16).rearrange(
                "p (c k) -> p c k", c=NC_
            )
            w1T = inb_sb[:, _B_W1 : _B_W1 + 1024].bitcast(f16).rearrange(
                "p (c o) -> p c o", c=NC_
            )
            brow = inb_sb[:, _B_BROW : _B_BROW + 256].bitcast(f16)
            ones_row = inb_sb[:, _B_ONES : _B_ONES + 128].bitcast(f16)

            # --- DMA observers: a tiny op per (engine, DMA half) so every
            # later instr carries at most one fresh wait. ---
            act_scr = consts.tile([128, 2], f32)
            nc.scalar.activation(
                out=act_scr[:, 0:1], in_=ina_sb[:, 0:1], func=Act.Identity,
                scale=one_col, bias=z_col,
            )
            nc.scalar.activation(
                out=act_scr[:, 1:2], in_=inb_sb[:, 0:1], func=Act.Identity,
                scale=one_col, bias=z_col,
            )
            dve_scr = consts.tile([128, 2], f32)
            nc.vector.tensor_copy(out=dve_scr[:, 0:1], in_=ina_sb[:, 0:1])
            nc.vector.tensor_copy(out=dve_scr[:, 1:2], in_=inb_sb[:, 0:1])
            pool_scr = consts.tile([128, 1], f32)
            nc.gpsimd.tensor_copy(out=pool_scr, in_=ina_sb[:, 0:1])
            dps = dum_ps.tile([1, 2], f32, tag="dummy")
            nc.tensor.matmul(
                dps[:, 0:1], lhsT=ina_sb[:, 0:1], rhs=ina_sb[:, 0:1],
                start=True, stop=True,
            )

            # --- q projection first (only needs DMA half A) ---
            qt_ps = qproj_ps.tile([128, NC_, TQ], f32)
            for co in range(NC_):
                for ci in range(NC_):
                    nc.tensor.matmul(
                        qt_ps[:, co, :],
                        lhsT=w2T[:, ci, co * 128 : (co + 1) * 128],
                        rhs=qT[:, ci, :],
                        start=(ci == 0),
                        stop=(ci == NC_ - 1),
                    )
            # PE observer for DMA half B, then k projection
            nc.tensor.matmul(
                dps[:, 1:2], lhsT=inb_sb[:, 0:1], rhs=inb_sb[:, 0:1],
                start=True, stop=True,
            )
            kt_ps = kproj_ps.tile([128, NC_, TK], f32)
            for co in range(NC_):
                for ci in range(NC_):
                    nc.tensor.matmul(
                        kt_ps[:, co, :],
                        lhsT=w1T[:, ci, co * 128 : (co + 1) * 128],
                        rhs=kT[:, ci, :],
                        start=(ci == 0),
                        stop=False,
                    )
                # bias: kt += b1+b2 via (b row)^T @ (ones row), K=1
                nc.tensor.matmul(
                    kt_ps[:, co, :],
                    lhsT=brow[0:1, co * 128 : (co + 1) * 128],
                    rhs=ones_row[0:1, 0:TK],
                    start=False,
                    stop=True,
                )

            # --- chain buffers: [128, lvl, chunk, T] f16 per (ladder, kind) ---
            sq = [chain.tile([128, NLVLS[i], NC_, TQ], f16, name=f"sq{i}") for i in range(NLAD)]
            cq = [chain.tile([128, NLVLS[i], NC_, TQ], f16, name=f"cq{i}") for i in range(NLAD)]
            sk = [chain.tile([128, NLVLS[i], NC_, TK], f16, name=f"sk{i}") for i in range(NLAD)]
            ck = [chain.tile([128, NLVLS[i], NC_, TK], f16, name=f"ck{i}") for i in range(NLAD)]
            lhsA = [chain.tile([128, NLVLS[i], NC_, TQ], f16, name=f"lhsA{i}") for i in range(NLAD)]
            lhsB = [chain.tile([128, NLVLS[i], NC_, TQ], f16, name=f"lhsB{i}") for i in range(NLAD)]

            def flat(t, lvl):
                return t[:, lvl].rearrange("p c t -> p (c t)")

            # --- base sins (ACT reads PSUM directly; batched over chunks) ---
            qf = qt_ps[:].rearrange("p c q -> p (c q)")
            kf = kt_ps[:].rearrange("p c k -> p (c k)")
            for bi in range(NLAD):
                om_col = scale_cols[:, bi : bi + 1]
                nc.scalar.activation(
                    out=flat(sq[bi], 0), in_=qf, func=Act.Sin, scale=om_col,
                    bias=z_col,
                )
                nc.scalar.activation(
                    out=flat(cq[bi], 0), in_=qf, func=Act.Sin, scale=om_col,
                    bias=hp_col,
                )
                nc.scalar.activation(
                    out=flat(sk[bi], 0), in_=kf, func=Act.Sin, scale=om_col,
                    bias=z_col,
                )
                nc.scalar.activation(
                    out=flat(ck[bi], 0), in_=kf, func=Act.Sin, scale=om_col,
                    bias=hp_col,
                )

            # --- doubling ladders ---
            def chain_level(lvl):
                for bi in range(NLAD):
                    if lvl >= NLVLS[bi]:
                        continue
                    for side, (s_, c_, tlen) in enumerate(
                        ((sq, cq, TQ), (sk, ck, TK))
                    ):
                        tau = scr.tile([128, NC_ * tlen], f16, tag=f"tau{side}")
                        nc.scalar.activation(
                            out=tau, in_=flat(s_[bi], lvl - 1), func=Act.Square,
                            scale=one_col, bias=z_col,
                        )
                        sobs_i = None
                        if lvl >= 2:
                            # DVE observer: absorb the Pool dep (c_{l-1}) so
                            # the sigma TT below carries a single wait.
                            sobs = scr.tile([1, 1], f16, tag="sobs")
                            sobs_i = nc.vector.tensor_copy(
                                out=sobs, in_=flat(c_[bi], lvl - 1)[0:1, 0:1]
                            )
                        tt_i = nc.vector.tensor_tensor(
                            out=flat(s_[bi], lvl),
                            in0=flat(s_[bi], lvl - 1),
                            in1=flat(c_[bi], lvl - 1),
                            op=Alu.mult,
                        )
                        if sobs_i is not None:
                            tile.add_dep_helper(
                                tt_i.ins, sobs_i.ins, sync=False,
                                reason="sigma TT right after its Pool-observer",
                            )
                        nc.gpsimd.tensor_scalar(
                            out=flat(c_[bi], lvl),
                            in0=tau,
                            scalar1=-float(2 ** (2 * (lvl - 1) + 1)),
                            scalar2=1.0,
                            op0=Alu.mult,
                            op1=Alu.add,
                        )

            scores_ps = score_ps.tile([128, TK], f32)
            n_mm = 2 * sum(NLVLS) * NC_
            mm_state = {"i": 0}

            def emit_level(lvl):
                """Fold lhs tiles for this level (fused v*ctilde TS) and issue
                its score matmuls."""
                for bi in range(NLAD):
                    if lvl >= NLVLS[bi]:
                        continue
                    ct = float(COEF[bi][lvl] * (2.0**lvl))
                    for co in range(NC_):
                        vcol = v_cols[:, co : co + 1]
                        eng = nc.vector if co < 2 else nc.gpsimd
                        eng.tensor_scalar(
                            out=lhsA[bi][:, lvl, co, :],
                            in0=sq[bi][:, lvl, co, :],
                            scalar1=vcol,
                            scalar2=ct,
                            op0=Alu.mult,
                            op1=Alu.mult,
                        )
                        eng.tensor_scalar(
                            out=lhsB[bi][:, lvl, co, :],
                            in0=cq[bi][:, lvl, co, :],
                            scalar1=vcol,
                            scalar2=ct,
                            op0=Alu.mult,
                            op1=Alu.mult,
                        )
                # PE gates: cover Pool (c of this level; ladder 1 c-TS is
                # emitted last) and, for level 0, ACT (base sins); the
                # matmuls then only carry the DVE wait.
                gpt = lhsB[NLAD - 1][:, lvl, NC_ - 1, 0:1]
                gp = gate_ps.tile([1, 1], f32, tag="gate")
                nc.tensor.matmul(gp, lhsT=gpt, rhs=gpt, start=True, stop=True)
                if lvl == 0:
                    ga = gate_ps.tile([1, 1], f32, tag="gate")
                    nc.tensor.matmul(
                        ga, lhsT=flat(ck[NLAD - 1], 0)[:, 0:1],
                        rhs=flat(ck[NLAD - 1], 0)[:, 0:1], start=True, stop=True,
                    )
                for bi in range(NLAD):
                    if lvl >= NLVLS[bi]:
                        continue
                    for co in range(NC_):
                        nc.tensor.matmul(
                            scores_ps,
                            lhsT=lhsA[bi][:, lvl, co, :],
                            rhs=ck[bi][:, lvl, co, :],
                            start=(mm_state["i"] == 0),
                            stop=(mm_state["i"] == n_mm - 1),
                        )
                        mm_state["i"] += 1
                        nc.tensor.matmul(
                            scores_ps,
                            lhsT=lhsB[bi][:, lvl, co, :],
                            rhs=sk[bi][:, lvl, co, :],
                            start=(mm_state["i"] == 0),
                            stop=(mm_state["i"] == n_mm - 1),
                        )
                        mm_state["i"] += 1

            emit_level(0)
            chain_level(1)
            emit_level(1)
            chain_level(2)
            emit_level(2)
            chain_level(3)
            emit_level(3)
            assert mm_state["i"] == n_mm

            scores_sb = outp.tile([128, TK], f32)
            nc.vector.tensor_copy(out=scores_sb, in_=scores_ps)
            nc.sync.dma_start(out=scores[:], in_=scores_sb)

    return nc


def _get_nc():
    if "nc" not in _cache:
        _cache["nc"] = _build_nc()
    return _cache["nc"]


def _make_in_maps(query, key, W1, b1, W2, b2, v):
    query = np.asarray(query, np.float32)
    key = np.asarray(key, np.float32)
    W1 = np.asarray(W1, np.float32)
    W2 = np.asarray(W2, np.float32)
    bsum = np.asarray(b1, np.float32) + np.asarray(b2, np.float32)
    v = np.asarray(v, np.float32)
    f16 = np.float16

    def part_major16(mT, n):
        # [D, n] -> f16 [128, NC_*n] with [p, c*n+j] = mT[c*128+p, j]
        return np.ascontiguousarray(
            mT.reshape(NC_, 128, n).transpose(1, 0, 2).reshape(128, NC_ * n)
        ).astype(f16)

    w2t = part_major16(W2.T, D).view(np.float32)  # [128, 1024]
    w1t = part_major16(W1.T, D).view(np.float32)
    brow = np.zeros((128, 512), f16)
    brow[0, :] = bsum.astype(f16)
    ones = np.zeros((128, 256), f16)
    ones[0, :] = 1.0
    scales = np.tile(np.array([[BASES[0], 0.0]], np.float32), (128, 1))  # [128,2]
    hp = np.full((128, 1), HALF_PI, np.float32)
    vc = np.ascontiguousarray(v.reshape(NC_, 128).T)  # [128, 4]
    zcol = np.zeros((128, 1), np.float32)
    onecol = np.ones((128, 1), np.float32)

    in_maps = []
    for b in range(B):
        qT = part_major16(query[b].T, TQ).view(np.float32)  # [128, 256]
        kT = part_major16(key[b].T, TK).view(np.float32)  # [128, 512]
        inpa = np.concatenate([qT, w2t, scales, hp, vc, zcol, onecol], axis=1)
        inpb = np.concatenate(
            [kT, w1t, brow.view(np.float32), ones.view(np.float32)], axis=1
        )
        assert inpa.shape == (128, _NCOLA), inpa.shape
        assert inpb.shape == (128, _NCOLB), inpb.shape
        in_maps.append(
            {
                "inpa": np.ascontiguousarray(inpa),
                "inpb": np.ascontiguousarray(inpb),
            }
        )
    return in_maps


def run(query, key, W1, b1, W2, b2, v, **run_kwargs):
    from concourse.bass_utils import run_bass_kernel_spmd

    nc = _get_nc()
    in_maps = _make_in_maps(query, key, W1, b1, W2, b2, v)
    res = run_bass_kernel_spmd(nc, in_maps, core_ids=list(range(B)), **run_kwargs)
    out = np.stack([r["scores"] for r in res.results]).astype(np.float32)
    return out, res


def kernel(query, key, W1, b1, W2, b2, v):
    out, _ = run(query, key, W1, b1, W2, b2, v)
    return out


# revision 7
# speedup vs baseline: 1.0544x; 1.0544x over previous
"""Bahdanau additive attention scores on 8 Trainium2 NeuronCores — v2.

reference:
    a = query @ W2.T                      # [B, Tq, D]
    bk = key @ W1.T + (b1 + b2)           # [B, Tk, D]
    scores[b,q,k] = sum_d v[d] * tanh(a[b,q,d] + bk[b,k,d])

Approximation: tanh(x) ~= sum_i c_i sin(om_i x) with om_i from a
frequency-doubling ladder {0.49, 0.98, 1.96, 3.92} (ridge-fit to the
Gaussian-weighted data range; end-to-end rel err ~5.5e-3, well under the
2e-2 gate).  Each sine of a sum splits separably:
    sin(om(a+b)) = sin(om a)cos(om b) + cos(om a)sin(om b)
so scores become 2*F=8 rank-D matmul accumulations per chunk — PE work —
and the only elementwise work is per-side sin/cos at 4 frequencies.

Per-side tables are built WITHOUT range reduction (the DVE has no float mod):
  - base sin/cos via one ACT Sin each (args |om0 x| <= 1.71; the cos
    phase-shift +pi/2 keeps args <= 3.28, inside the HW table's accurate
    range, measured good to ~3.3),
  - each doubling level: tau = sigma_{l-1}^2 (ACT Square),
    sigma_l = sigma_{l-1} * c_{l-1} (DVE tensor_tensor),
    c_l = 1 - 2^(2l-1) tau (Pool tensor_scalar).
    sigma_l stores sin/2^l; the 2^l rides the fitted coefficient.

The k-side bias (b1+b2) is added inside the PE projection via a K=1 matmul
(bias row x ones row), so the k-side Sin instructions batch all 4 feature
chunks.  The input rides two parallel DMA queues (q-projection operands on
one, k-side on the other).  Score matmuls are issued per ladder level, so
level-l matmuls overlap the level-l+1 chain work.

Sharding: data-parallel over batch B=8 -> 1 batch per core; weights
replicated.  D=512 is split into 4 partition chunks of 128; PE projects
q/k with f16 weights straight into PSUM and ACT reads PSUM directly.
"""

import sys

sys.path.insert(0, "/opt/trn_rl_repo")

import numpy as np

B, TQ, TK, D = 8, 128, 256, 512
NC_ = 4  # 128-wide feature chunks
BASES = (0.49,)
NLVLS = (4,)  # ladder levels per base (freqs base * 2^l)
NLAD = len(BASES)
# ridge fit (alpha=3e-3, weighted by empirical |a+b| density, S=6.12):
COEF = ((1.06213, 0.17335, 0.13537, 0.01141),)
HALF_PI = float(np.pi / 2)

_cache = {}


def _patch_drain(tile):
    """Split kernel-tail drain waits over several SP no-ops (<=3 waits each) to
    stay within the ISA's per-instruction sync-wait budget."""
    from concourse.vector_clock import VectorClock, ScopedClock

    if getattr(tile.TileContext, "_drain_patched", False):
        return

    def _drain_and_barrier(self, tick_clock, wait_clock):
        gc = tick_clock.global_clock
        pending = [(p, t) for p, t in enumerate(gc) if t > 0]
        for i in range(0, len(pending), 1):
            chunk = pending[i : i + 1]
            nop = self.nc.sync.nop(nofuse=True, hint="drain_split")
            req = VectorClock()
            for p, t in chunk:
                req.require_at_least(p, t)
            wait_clock.add_sem_waits(nop.ins, ScopedClock({None: req}))
        self.nc.sync.drain()
        self.nc.all_engine_barrier()
        assert self.sems is not None
        popped = self.nc._tile_sem_poison_stack.pop()
        assert popped is self._sem_poison
        self.nc.clear_and_free_semaphores(list(self.sems.allocated().values()))
        self.nc.all_engine_barrier()

    tile.TileContext._drain_and_barrier = _drain_and_barrier
    tile.TileContext._drain_patched = True


# packed-input layout, two DMA halves (f32 word columns)
# half A (q-projection + scalars): qT (256), w2T (1024), scales (2), hp (1),
#   v (4)  -> 1287 cols
# half B (k-side): kT (512), w1T (1024), brow (256), ones (128) -> 1920 cols
_A_QT = 0
_A_W2 = _A_QT + 256
_A_SCALE = _A_W2 + 1024
_A_HP = _A_SCALE + 2
_A_V = _A_HP + 1
_A_Z = _A_V + 4  # zero col (explicit activation bias; const-AP table is skipped)
_A_ONE = _A_Z + 1  # one col (explicit activation scale)
_NCOLA = _A_ONE + 1
_B_KT = 0
_B_W1 = _B_KT + 512
_B_BROW = _B_W1 + 1024
_B_ONES = _B_BROW + 256
_NCOLB = _B_ONES + 128


def _build_nc():
    import concourse.bass as bass
    import concourse.mybir as mybir
    import concourse.tile as tile

    _patch_drain(tile)
    f32 = mybir.dt.float32
    f16 = mybir.dt.float16
    Alu = mybir.AluOpType
    Act = mybir.ActivationFunctionType

    # Skip the const-AP memsets and the init all-engine barrier: every
    # activation below passes explicit scale/bias column APs, so the const-AP
    # table is never read, and removing the barrier lets the input DMA issue
    # as soon as the SP engine finishes its own bootstrap.
    orig_bar = bass.Bass.all_engine_barrier
    orig_memset = bass.BassSharedVectorInterface.memset
    bass.Bass.all_engine_barrier = lambda self: None
    bass.BassSharedVectorInterface.memset = lambda self, ap, c: None
    try:
        nc = bass.Bass()
    finally:
        bass.Bass.all_engine_barrier = orig_bar
        bass.BassSharedVectorInterface.memset = orig_memset
    inpa = nc.dram_tensor("inpa", [128, _NCOLA], f32, kind="ExternalInput")
    inpb = nc.dram_tensor("inpb", [128, _NCOLB], f32, kind="ExternalInput")
    scores = nc.dram_tensor("scores", [TQ, TK], f32, kind="ExternalOutput")

    with tile.TileContext(nc) as tc:
        with (
            tc.tile_pool(name="consts", bufs=1) as consts,
            tc.tile_pool(name="qproj_ps", bufs=1, space="PSUM") as qproj_ps,
            tc.tile_pool(name="kproj_ps", bufs=1, space="PSUM") as kproj_ps,
            tc.tile_pool(name="score_ps", bufs=1, space="PSUM") as score_ps,
            tc.tile_pool(name="dum_ps", bufs=1, space="PSUM") as dum_ps,
            tc.tile_pool(name="gate_ps", bufs=1, space="PSUM") as gate_ps,
            tc.tile_pool(name="chain", bufs=1) as chain,
            tc.tile_pool(name="scr", bufs=8) as scr,
            tc.tile_pool(name="outp", bufs=1) as outp,
        ):
            ina_sb = consts.tile([128, _NCOLA], f32)
            nc.sync.dma_start(out=ina_sb, in_=inpa[:])
            inb_sb = consts.tile([128, _NCOLB], f32)
            nc.sync.dma_start(out=inb_sb, in_=inpb[:])

            qT = ina_sb[:, _A_QT : _A_QT + 256].bitcast(f16).rearrange(
                "p (c q) -> p c q", c=NC_
            )
            w2T = ina_sb[:, _A_W2 : _A_W2 + 1024].bitcast(f16).rearrange(
                "p (c o) -> p c o", c=NC_
            )
            scale_cols = ina_sb[:, _A_SCALE : _A_SCALE + 2]
            hp_col = ina_sb[:, _A_HP : _A_HP + 1]
            v_cols = ina_sb[:, _A_V : _A_V + 4]
            z_col = ina_sb[:, _A_Z : _A_Z + 1]
            one_col = ina_sb[:, _A_ONE : _A_ONE + 1]
            kT = inb_sb[:, _B_KT : _B_KT + 512].bitcast(f16).rearrange(
                "p (c k) -> p c k", c=NC_
            )
            w1T = inb_sb[:, _B_W1 : _B_W1 + 1024].bitcast(f16).rearrange(
                "p (c o) -> p c o", c=NC_
            )
            brow = inb_sb[:, _B_BROW : _B_BROW + 256].bitcast(f16)
            ones_row = inb_sb[:, _B_ONES : _B_ONES + 128].bitcast(f16)

            # --- DMA observers: a tiny op per (engine, DMA half) so every
            # later instr carries at most one fresh wait. ---
            act_scr = consts.tile([128, 2], f32)
            nc.scalar.activation(
                out=act_scr[:, 0:1], in_=ina_sb[:, 0:1], func=Act.Identity,
                scale=one_col, bias=z_col,
            )
            nc.scalar.activation(
                out=act_scr[:, 1:2], in_=inb_sb[:, 0:1], func=Act.Identity,
                scale=one_col, bias=z_col,
            )
            dve_scr = consts.tile([128, 2], f32)
            nc.vector.tensor_copy(out=dve_scr[:, 0:1], in_=ina_sb[:, 0:1])
            nc.vector.tensor_copy(out=dve_scr[:, 1:2], in_=inb_sb[:, 0:1])
            pool_scr = consts.tile([128, 1], f32)
            nc.gpsimd.tensor_copy(out=pool_scr, in_=ina_sb[:, 0:1])
            dps = dum_ps.tile([1, 2], f32, tag="dummy")
            nc.tensor.matmul(
                dps[:, 0:1], lhsT=ina_sb[:, 0:1], rhs=ina_sb[:, 0:1],
                start=True, stop=True,
            )

            # --- q projection first (only needs DMA half A) ---
            qt_ps = qproj_ps.tile([128, NC_, TQ], f32)
            for co in range(NC_):
                for ci in range(NC_):
                    nc.tensor.matmul(
                        qt_ps[:, co, :],
                        lhsT=w2T[:, ci, co * 128 : (co + 1) * 128],
                        rhs=qT[:, ci, :],
                        start=(ci == 0),
                        stop=(ci == NC_ - 1),
                    )
            # PE observer for DMA half B, then k projection
            nc.tensor.matmul(
                dps[:, 1:2], lhsT=inb_sb[:, 0:1], rhs=inb_sb[:, 0:1],
                start=True, stop=True,
            )
            kt_ps = kproj_ps.tile([128, NC_, TK], f32)
            for co in range(NC_):
                for ci in range(NC_):
                    nc.tensor.matmul(
                        kt_ps[:, co, :],
                        lhsT=w1T[:, ci, co * 128 : (co + 1) * 128],
                        rhs=kT[:, ci, :],
                        start=(ci == 0),
                        stop=False,
                    )
                # bias: kt += b1+b2 via (b row)^T @ (ones row), K=1
                nc.tensor.matmul(
                    kt_ps[:, co, :],
                    lhsT=brow[0:1, co * 128 : (co + 1) * 128],
                    rhs=ones_row[0:1, 0:TK],
                    start=False,
                    stop=True,
                )

            # --- chain buffers: [128, lvl, chunk, T] f16 per (ladder, kind) ---
            sq = [chain.tile([128, NLVLS[i], NC_, TQ], f16, name=f"sq{i}") for i in range(NLAD)]
            cq = [chain.tile([128, NLVLS[i], NC_, TQ], f16, name=f"cq{i}") for i in range(NLAD)]
            sk = [chain.tile([128, NLVLS[i], NC_, TK], f16, name=f"sk{i}") for i in range(NLAD)]
            ck = [chain.tile([128, NLVLS[i], NC_, TK], f16, name=f"ck{i}") for i in range(NLAD)]
            lhsA = [chain.tile([128, NLVLS[i], NC_, TQ], f16, name=f"lhsA{i}") for i in range(NLAD)]
            lhsB = [chain.tile([128, NLVLS[i], NC_, TQ], f16, name=f"lhsB{i}") for i in range(NLAD)]

            def flat(t, lvl):
                return t[:, lvl].rearrange("p c t -> p (c t)")

            # --- base sins (ACT reads PSUM directly; batched over chunks) ---
            qf = qt_ps[:].rearrange("p c q -> p (c q)")
            kf = kt_ps[:].rearrange("p c k -> p (c k)")
            for bi in range(NLAD):
                om_col = scale_cols[:, bi : bi + 1]
                nc.scalar.activation(
                    out=flat(sq[bi], 0), in_=qf, func=Act.Sin, scale=om_col,
                    bias=z_col,
                )
                nc.scalar.activation(
                    out=flat(cq[bi], 0), in_=qf, func=Act.Sin, scale=om_col,
                    bias=hp_col,
                )
                nc.scalar.activation(
                    out=flat(sk[bi], 0), in_=kf, func=Act.Sin, scale=om_col,
                    bias=z_col,
                )
                nc.scalar.activation(
                    out=flat(ck[bi], 0), in_=kf, func=Act.Sin, scale=om_col,
                    bias=hp_col,
                )

            # --- doubling ladders ---
            def chain_level(lvl):
                for bi in range(NLAD):
                    if lvl >= NLVLS[bi]:
                        continue
                    for side, (s_, c_, tlen) in enumerate(
                        ((sq, cq, TQ), (sk, ck, TK))
                    ):
                        tau = scr.tile([128, NC_ * tlen], f16, tag=f"tau{side}")
                        nc.scalar.activation(
                            out=tau, in_=flat(s_[bi], lvl - 1), func=Act.Square,
                            scale=one_col, bias=z_col,
                        )
                        sobs_i = None
                        if lvl >= 2:
                            # DVE observer: absorb the Pool dep (c_{l-1}) so
                            # the sigma TT below carries a single wait.
                            sobs = scr.tile([1, 1], f16, tag="sobs")
                            sobs_i = nc.vector.tensor_copy(
                                out=sobs, in_=flat(c_[bi], lvl - 1)[0:1, 0:1]
                            )
                        tt_i = nc.vector.tensor_tensor(
                            out=flat(s_[bi], lvl),
                            in0=flat(s_[bi], lvl - 1),
                            in1=flat(c_[bi], lvl - 1),
                            op=Alu.mult,
                        )
                        if sobs_i is not None:
                            tile.add_dep_helper(
                                tt_i.ins, sobs_i.ins, sync=False,
                                reason="sigma TT right after its Pool-observer",
                            )
                        nc.gpsimd.tensor_scalar(
                            out=flat(c_[bi], lvl),
                            in0=tau,
                            scalar1=-float(2 ** (2 * (lvl - 1) + 1)),
                            scalar2=1.0,
                            op0=Alu.mult,
                            op1=Alu.add,
                        )

            scores_ps = score_ps.tile([128, TK], f32)
            n_mm = 2 * sum(NLVLS) * NC_
            mm_state = {"i": 0}

            def emit_level(lvl):
                """Fold lhs tiles for this level (fused v*ctilde TS) and issue
                its score matmuls."""
                for bi in range(NLAD):
                    if lvl >= NLVLS[bi]:
                        continue
                    ct = float(COEF[bi][lvl] * (2.0**lvl))
                    for co in range(NC_):
                        vcol = v_cols[:, co : co + 1]
                        eng = nc.vector if co < 2 else nc.gpsimd
                        eng.tensor_scalar(
                            out=lhsA[bi][:, lvl, co, :],
                            in0=sq[bi][:, lvl, co, :],
                            scalar1=vcol,
                            scalar2=ct,
                            op0=Alu.mult,
                            op1=Alu.mult,
                        )
                        eng.tensor_scalar(
                            out=lhsB[bi][:, lvl, co, :],
                            in0=cq[bi][:, lvl, co, :],
                            scalar1=vcol,
                            scalar2=ct,
                            op0=Alu.mult,
                            op1=Alu.mult,
                        )
                # PE gates: cover Pool (c of this level; ladder 1 c-TS is
                # emitted last) and, for level 0, ACT (base sins); the
                # matmuls then only carry the DVE wait.
                gpt = lhsB[NLAD - 1][:, lvl, NC_ - 1, 0:1]
                gp = gate_ps.tile([1, 1], f32, tag="gate")
                nc.tensor.matmul(gp, lhsT=gpt, rhs=gpt, start=True, stop=True)
                if lvl == 0:
                    ga = gate_ps.tile([1, 1], f32, tag="gate")
                    nc.tensor.matmul(
                        ga, lhsT=flat(ck[NLAD - 1], 0)[:, 0:1],
                        rhs=flat(ck[NLAD - 1], 0)[:, 0:1], start=True, stop=True,
                    )
                for bi in range(NLAD):
                    if lvl >= NLVLS[bi]:
                        continue
                    for co in range(NC_):
                        nc.tensor.matmul(
                            scores_ps,
                            lhsT=lhsA[bi][:, lvl, co, :],
                            rhs=ck[bi][:, lvl, co, :],
                            start=(mm_state["i"] == 0),
                            stop=(mm_state["i"] == n_mm - 1),
                        )
                        mm_state["i"] += 1
                        nc.tensor.matmul(
                            scores_ps,
                            lhsT=lhsB[bi][:, lvl, co, :],
                            rhs=sk[bi][:, lvl, co, :],
                            start=(mm_state["i"] == 0),
                            stop=(mm_state["i"] == n_mm - 1),
                        )
                        mm_state["i"] += 1

            emit_level(0)
            chain_level(1)
            emit_level(1)
            chain_level(2)
            emit_level(2)
            chain_level(3)
            emit_level(3)
            assert mm_state["i"] == n_mm

            scores_sb = outp.tile([128, TK], f32)
            nc.vector.tensor_copy(out=scores_sb, in_=scores_ps)
            nc.sync.dma_start(out=scores[:], in_=scores_sb)

    return nc


def _get_nc():
    if "nc" not in _cache:
        _cache["nc"] = _build_nc()
    return _cache["nc"]


def _make_in_maps(query, key, W1, b1, W2, b2, v):
    query = np.asarray(query, np.float32)
    key = np.asarray(key, np.float32)
    W1 = np.asarray(W1, np.float32)
    W2 = np.asarray(W2, np.float32)
    bsum = np.asarray(b1, np.float32) + np.asarray(b2, np.float32)
    v = np.asarray(v, np.float32)
    f16 = np.float16

    def part_major16(mT, n):
        # [D, n] -> f16 [128, NC_*n] with [p, c*n+j] = mT[c*128+p, j]
        return np.ascontiguousarray(
            mT.reshape(NC_, 128, n).transpose(1, 0, 2).reshape(128, NC_ * n)
        ).astype(f16)

    w2t = part_major16(W2.T, D).view(np.float32)  # [128, 1024]
    w1t = part_major16(W1.T, D).view(np.float32)
    brow = np.zeros((128, 512), f16)
    brow[0, :] = bsum.astype(f16)
    ones = np.zeros((128, 256), f16)
    ones[0, :] = 1.0
    scales = np.tile(np.array([[BASES[0], 0.0]], np.float32), (128, 1))  # [128,2]
    hp = np.full((128, 1), HALF_PI, np.float32)
    vc = np.ascontiguousarray(v.reshape(NC_, 128).T)  # [128, 4]
    zcol = np.zeros((128, 1), np.float32)
    onecol = np.ones((128, 1), np.float32)

    in_maps = []
    for b in range(B):
        qT = part_major16(query[b].T, TQ).view(np.float32)  # [128, 256]
        kT = part_major16(key[b].T, TK).view(np.float32)  # [128, 512]
        inpa = np.concatenate([qT, w2t, scales, hp, vc, zcol, onecol], axis=1)
        inpb = np.concatenate(
            [kT, w1t, brow.view(np.float32), ones.view(np.float32)], axis=1
        )
        assert inpa.shape == (128, _NCOLA), inpa.shape
        assert inpb.shape == (128, _NCOLB), inpb.shape
        in_maps.append(
            {
                "inpa": np.ascontiguousarray(inpa),
                "inpb": np.ascontiguousarray(inpb),
            }
        )
    return in_maps


def run(query, key, W1, b1, W2, b2, v, **run_kwargs):
    from concourse.bass_utils import run_bass_kernel_spmd

    nc = _get_nc()
    in_maps = _make_in_maps(query, key, W1, b1, W2, b2, v)
    res = run_bass_kernel_spmd(nc, in_maps, core_ids=list(range(B)), **run_kwargs)
    out = np.stack([r["scores"] for r in res.results]).astype(np.float32)
    return out, res


def kernel(query, key, W1, b1, W2, b2, v):
    out, _ = run(query, key, W1, b1, W2, b2, v)
    return out
